# revision 5
# baseline (speedup 1.0000x reference)
"""Causal single-head attention on 8 Trainium2 NeuronCores.

Problem: embedding_word [4, 2048, 1024] fp32; w_q/w_k/w_v [1024, 1024] fp32.
  q = x @ w_q; k = x @ w_k; v = x @ w_v
  out = softmax(causal_mask(q k^T) / 32) @ v          per batch.

Sharding: 4 batches x 2 key-shards = 8 cores (SPMD, one program).
Core (b, p) handles batch b and the interleaved key blocks
{128*(2i+p) .. +128 : i in 0..7} (1024 keys), for ALL 2048 query rows,
producing the *unnormalized* attention output sum_s exp(score) * v[s] and
the per-row sum of exp.  Host combines the two key-shards per batch:
  out = (u_p0 + u_p1) / (s_p0 + s_p1).
Scores are bounded (|score/32| < ~2 for these randn/uniform inputs), so
softmax without max-subtraction is numerically safe and the partial sums
combine linearly.

All matmuls run in bf16 (fp32 PSUM accumulation).  The host pre-transposes
x (and pre-gathers the key shard) so the device needs no transposes:
  qT[dq, t] = wq^T  xT            (lhsT=wq,   rhs=xT)
  kT[dq, s] = wk^T  xkvT          (lhsT=wk,   rhs=xkvT)
  v [s, dv] = xkvT^T wv           (lhsT=xkvT, rhs=wv)
  scT[s, t] = kT^T qT             (lhsT=kT,   rhs=qT)
  e = exp(scT/32) * mask          (diag slot only)
  sums[1,t] += ones^T e           (lhsT=ones, rhs=e)
  u[t, dv]  += e^T v              (lhsT=e,    rhs=v)
"""

import numpy as np
import ml_dtypes

try:
    import concourse.bass as bass  # noqa: F401
except ImportError:  # pragma: no cover
    import sys

    sys.path.insert(0, "/opt/trn_rl_repo")
    import concourse.bass as bass  # noqa: F401

from contextlib import ExitStack

import concourse.tile as tile
from concourse import bacc, mybir
from concourse.bass_utils import run_bass_kernel_spmd

B = 4
T = 2048
D = 1024
P = 128
KT = D // P  # 8 contraction subtiles of 128
NSLOT = 8  # key slots per core (each 128 packed keys)
TJ = 256  # query-row tile
NJ = T // TJ  # 8 row tiles
BF16 = mybir.dt.bfloat16
F32 = mybir.dt.float32
SCALE = 1.0 / 32.0  # 1/sqrt(d_q)

_NC_CACHE = {}


def _build_program():
    nc = bacc.Bacc(
        "TRN2",
        target_bir_lowering=False,
        debug=False,
        enable_asserts=False,
        num_devices=8,
    )
    xt = nc.dram_tensor("xt", [D, T], BF16, kind="ExternalInput").ap()
    xkvt = nc.dram_tensor("xkvt", [D, NSLOT * P], BF16, kind="ExternalInput").ap()
    wq = nc.dram_tensor("wq", [D, D], BF16, kind="ExternalInput").ap()
    wk = nc.dram_tensor("wk", [D, D], BF16, kind="ExternalInput").ap()
    wv = nc.dram_tensor("wv", [D, D], BF16, kind="ExternalInput").ap()
    mask = nc.dram_tensor("mask", [P, TJ], BF16, kind="ExternalInput").ap()
    out_u = nc.dram_tensor("out_u", [T, D], F32, kind="ExternalOutput").ap()
    sums = nc.dram_tensor("sums", [NJ, TJ], F32, kind="ExternalOutput").ap()

    with tile.TileContext(nc) as tc, ExitStack() as ctx:
        _emit(ctx, tc, xt, xkvt, wq, wk, wv, mask, out_u, sums)
    nc.compile()
    return nc


def _emit(ctx, tc, xt, xkvt, wq, wk, wv, mask, out_u, sums):
    nc = tc.nc

    const = ctx.enter_context(tc.tile_pool(name="const", bufs=1))
    big = ctx.enter_context(tc.tile_pool(name="big", bufs=1))
    work = ctx.enter_context(tc.tile_pool(name="work", bufs=12))
    outp = ctx.enter_context(tc.tile_pool(name="outp", bufs=6))
    ps_w = ctx.enter_context(tc.tile_pool(name="ps_w", bufs=3, space="PSUM"))
    ps_av = ctx.enter_context(tc.tile_pool(name="ps_av", bufs=4, space="PSUM"))
    ps_s = ctx.enter_context(tc.tile_pool(name="ps_s", bufs=1, space="PSUM"))

    # Persistent SBUF tensors (layout [128 partitions, outer, free]).
    xt_sb = big.tile([P, KT, T], BF16)  # x^T        [dm_p, dm_o, t]
    xkv_sb = big.tile([P, KT, NSLOT * P], BF16)  # gathered x^T for keys
    wq_sb = big.tile([P, KT, D], BF16)
    wk_sb = big.tile([P, KT, D], BF16)
    wv_sb = big.tile([P, KT, D], BF16)
    qt_sb = big.tile([P, KT, T], BF16)  # q^T        [dq_p, dq_o, t]
    kt_sb = big.tile([P, KT, NSLOT * P], BF16)  # k^T   [dq_p, dq_o, s]
    v_sb = big.tile([P, NSLOT, D], BF16)  # v          [s_p,  s_o,  dv]
    mask_sb = const.tile([P, TJ], BF16)
    ones_sb = const.tile([P, 1], BF16)

    nc.vector.memset(ones_sb[:], 1.0)
    nc.sync.dma_start(mask_sb[:], mask[:])
    # One big DMA per tensor (a single InstDMACopy fans out over all 16 SDMA
    # engines at ~HBM rate); FIFO order on the sync ring = dependency order,
    # so the K-projection's inputs land first.
    nc.sync.dma_start(wk_sb[:], wk.rearrange("(o p) n -> p o n", p=P))
    nc.sync.dma_start(xkv_sb[:], xkvt.rearrange("(o p) n -> p o n", p=P))
    nc.sync.dma_start(wv_sb[:], wv.rearrange("(o p) n -> p o n", p=P))
    nc.sync.dma_start(wq_sb[:], wq.rearrange("(o p) n -> p o n", p=P))
    nc.sync.dma_start(xt_sb[:], xt.rearrange("(o p) n -> p o n", p=P))

    def proj(lhs_sb, rhs_sb, out_sb, m_range, n_range):
        # out[m*128 block, n*512 block] = lhs^T @ rhs, contracting over dm.
        for m in range(m_range):
            for n in range(n_range):
                ps = ps_w.tile([P, 512], F32, tag="ps_work")
                for kt in range(KT):
                    nc.tensor.matmul(
                        ps[:],
                        lhs_sb[:, kt, m * P : (m + 1) * P],
                        rhs_sb[:, kt, n * 512 : (n + 1) * 512],
                        start=(kt == 0),
                        stop=(kt == KT - 1),
                    )
                nc.vector.tensor_copy(out_sb[:, m, n * 512 : (n + 1) * 512], ps[:])

    proj(wk_sb, xkv_sb, kt_sb, KT, 2)  # k^T
    proj(xkv_sb, wv_sb, v_sb, NSLOT, 2)  # v
    proj(wq_sb, xt_sb, qt_sb, KT, 4)  # q^T

    # Attention: row tiles of 256; slot i holds this core's packed keys
    # [128i, 128(i+1)), covering original key block 2i+p.  Tile J needs
    # slots 0..J; slot J is the diagonal block (mask applied).  Descending J
    # so the tail of the kernel drains the cheapest tiles.
    for J in reversed(range(NJ)):
        t0 = J * TJ
        av_ps = [
            [
                ps_av.tile([P, 512], F32, tag="ps_av", name=f"av_{J}_{c}_{h}")
                for h in range(2)
            ]
            for c in range(2)
        ]
        sums_ps = ps_s.tile([1, TJ], F32, tag="ps_sums")
        for i in range(J + 1):
            sc = ps_w.tile([P, TJ], F32, tag="ps_work")
            for kt in range(KT):
                nc.tensor.matmul(
                    sc[:],
                    kt_sb[:, kt, i * P : (i + 1) * P],
                    qt_sb[:, kt, t0 : t0 + TJ],
                    start=(kt == 0),
                    stop=(kt == KT - 1),
                )
            e = work.tile([P, TJ], BF16, tag="exp")
            nc.scalar.activation(
                e[:], sc[:], mybir.ActivationFunctionType.Exp, scale=SCALE
            )
            if i == J:
                nc.vector.tensor_tensor(e[:], e[:], mask_sb[:], mybir.AluOpType.mult)
            nc.tensor.matmul(
                sums_ps[:], ones_sb[:], e[:], start=(i == 0), stop=(i == J)
            )
            for c in range(2):
                for dvh in range(2):
                    nc.tensor.matmul(
                        av_ps[c][dvh][:],
                        e[:, c * P : (c + 1) * P],
                        v_sb[:, i, dvh * 512 : (dvh + 1) * 512],
                        start=(i == 0),
                        stop=(i == J),
                    )
        s_sb = outp.tile([1, TJ], F32, tag="sums_sb")
        nc.vector.tensor_copy(s_sb[:], sums_ps[:])
        nc.sync.dma_start(sums[J : J + 1, :], s_sb[:])
        for c in range(2):
            for dvh in range(2):
                o_sb = outp.tile([P, 512], F32, tag="o_sb")
                nc.vector.tensor_copy(o_sb[:], av_ps[c][dvh][:])
                nc.sync.dma_start(
                    out_u[t0 + c * P : t0 + (c + 1) * P, dvh * 512 : (dvh + 1) * 512],
                    o_sb[:],
                )


def _shard_inputs(x, wq, wk, wv):
    bf = ml_dtypes.bfloat16
    wq_b = np.ascontiguousarray(wq.astype(bf))
    wk_b = np.ascontiguousarray(wk.astype(bf))
    wv_b = np.ascontiguousarray(wv.astype(bf))
    in_maps = []
    for b in range(B):
        xb = x[b]  # [T, D] fp32
        xt = np.ascontiguousarray(xb.T.astype(bf))  # [D, T]
        for p in range(2):
            rows = np.concatenate(
                [xb[P * (2 * i + p) : P * (2 * i + p) + P] for i in range(NSLOT)], 0
            )  # [1024, D] the key shard
            xkvt = np.ascontiguousarray(rows.T.astype(bf))  # [D, 1024]
            m = (
                np.arange(TJ)[None, :] - np.arange(P)[:, None] >= P * p
            ).astype(bf)  # [128, 256] keep mask for the diagonal slot
            in_maps.append(
                {
                    "xt": xt,
                    "xkvt": xkvt,
                    "wq": wq_b,
                    "wk": wk_b,
                    "wv": wv_b,
                    "mask": np.ascontiguousarray(m),
                }
            )
    return in_maps


def run(embedding_word, w_q, w_k, w_v, trace=False, **spmd_kwargs):
    x = np.asarray(embedding_word, dtype=np.float32)
    assert x.shape == (B, T, D), x.shape
    if "nc" not in _NC_CACHE:
        _NC_CACHE["nc"] = _build_program()
    nc = _NC_CACHE["nc"]
    in_maps = _shard_inputs(
        x,
        np.asarray(w_q, np.float32),
        np.asarray(w_k, np.float32),
        np.asarray(w_v, np.float32),
    )
    res = run_bass_kernel_spmd(
        nc, in_maps, core_ids=list(range(8)), trace=trace, **spmd_kwargs
    )
    out = np.empty((B, T, D), np.float32)
    for b in range(B):
        u0 = res.results[2 * b]["out_u"]
        u1 = res.results[2 * b + 1]["out_u"]
        s0 = res.results[2 * b]["sums"].reshape(T)
        s1 = res.results[2 * b + 1]["sums"].reshape(T)
        out[b] = (u0 + u1) / (s0 + s1)[:, None]
    return out, res


def kernel(embedding_word, w_q, w_k, w_v):
    out, _ = run(embedding_word, w_q, w_k, w_v)
    return out
